# revision 77
# baseline (speedup 1.0000x reference)
"""Anisotropic Gaussian filter on 8 TRN2 NeuronCores (Bass/Tile).

Math per pixel p (global g = b*HW+p), window (i,j) in 7x7, r=3:
  dx = x[b,r,j,p] - x[b,i,j,p];  dy = x[b,i,r,p] - x[b,i,j,p]
  sx = sigx*|dx|, sy = sigy*|dy|
  arg = -0.5(sx^2 - 2 th sx sy + sy^2) - m*(0.5/sigr^2)
      = -[(u-v)^2 + uv*2(1-th) + m*spv],  u = sx/sqrt2, v = sy/sqrt2
  k = exp(arg);  out = sum_ij(k*x)/sum_ij(k)

v2 dataflow (window-on-partition, 98 = 2 batches x 49; T = 512-pixel
quanta, super-tiles of 2 quanta = 1 iteration):
  - HOST folds sigma scaling into two streamed copies of x:
      xsx = x*(sigx/sqrt2), xsy = x*(sigy/sqrt2)  [98, HWC] f16
    so no on-chip per-pixel coefficient scaling of squares is needed.
    Only ONE broadcast row remains: trow = 2(1-theta) (49-partition
    broadcast DMA), vs 3 rows in v1 (-13MB DMA).
  - x f16 [98, HWC] SBUF-resident (8 chunks, streamed across the run)
    for the k*x product.
  - Retimed 4-deep software pipeline; iteration u emits (in this order,
    so ACT's in-order stream runs exp BEFORE the new drains):
      E(u-3): ACT Exp(acc)-> ktw [k0|k1]; Pool w = k*x -> [w0|w1]
      A(u):   stream DMAs; PE dxs/dys matmuls -> e12 [98,4T] PSUM;
              ONE ACT Abs drain (permuted out AP) -> a12 [u0|u1|v0|v1]
      B(u-2): DVE 2T-wide d=u-v, uv=u*v, dsq=d*d, h2=uv*trow; PE
              acc[:,qT] = I @ dsq_q + [I;wsp] @ h2_q (the [100,98]
              stacked lhsT adds m*spv; spv rides rows 98:100 of the
              per-group h2 tile, DMA'd at group load)
      R(u-4): PE band-matrix reduction matmuls: quantum t accumulates
              rows t / 64+t of a [128,T] PSUM pair over all 64 quanta
    Steady state is dual-bound at ~3.1us/iteration: ACT work (drain 1.9
    + exp 1.0) coincides with the e12 PSUM recycle loop (drain -> sem ->
    4 A-matmuls -> sem). Drain phase: the final two accs reuse the dead
    e12 banks so the exps run back-to-back on ACT; tail: approx-
    reciprocal + mult + ONE merged output DMA ("b (t f) -> (b t) f").
  - Startup: PE p-state warm-up matmuls; ONE packed weight DMA via Pool
    SWDGE (off the shared HWDGE pipe); JIT group prefetch (g at 2g-3).
PSUM: e12 [98,4T] x1 + acc [98,2T] x1 + red 2x[128,T] = 8 banks.
"""

from contextlib import ExitStack

import numpy as np

B, KS, HW = 2, 7, 262144
NCORES = 8
HWC = HW // NCORES          # pixels per core per batch (32768)
T = 512                     # pixels per tile
NT = HWC // T               # tiles per core (64)
NH = NT // 32               # reduction halves (2)
P98 = 2 * KS * KS           # 98
R = KS // 2

_compiled = {}


def _build_weights():
    iy, ix = np.meshgrid(np.arange(KS), np.arange(KS), indexing="ij")
    m = ((iy - R) ** 2 + (ix - R) ** 2).reshape(-1).astype(np.float32)  # (49,)

    wdx = np.zeros((P98, P98), np.float32)   # dxs = xsx[center row] - xsx
    wdy = np.zeros((P98, P98), np.float32)
    # stacked lhsT [100, 98]: rows 0:98 = I (plain identity adds),
    # rows 98:100 = wsp (m broadcast: acc[o] += m[o] * spv[batch(o)])
    idsp = np.zeros((100, P98), np.float32)
    idsp[0:P98, :] = np.eye(P98, dtype=np.float32)
    for b in range(2):
        o = 49 * b
        idsp[P98 + b, o:o + 49] = m
        for w in range(49):
            i, j = divmod(w, KS)
            wdx[o + R * KS + j, o + w] += 1.0
            wdx[o + w, o + w] -= 1.0
            wdy[o + i * KS + R, o + w] += 1.0
            wdy[o + w, o + w] -= 1.0

    # band matrix for shifted reduction columns: slice [:, 63-t : 191-t]
    # gives ones at out-row t (batch0) / 64+t (batch1) for quantum t
    bandw = np.zeros((P98, 191), np.float32)
    bandw[0:49, 63] = 1.0
    bandw[49:98, 127] = 1.0

    # pack: [0:98] wdx | [98:196] wdy | [196:294] idsp | [294:485] bandw
    wall = np.zeros((100, 485), np.float32)
    wall[0:P98, 0:P98] = wdx
    wall[0:P98, P98:2 * P98] = wdy
    wall[:, 2 * P98:2 * P98 + P98] = idsp
    wall[0:P98, 294:485] = bandw
    return wall


def _build_nc():
    import os

    import concourse.bacc as bacc
    import concourse.tile as tile
    from concourse import mybir

    f32 = mybir.dt.float32
    f16 = mybir.dt.float16
    Alu = mybir.AluOpType
    Act = mybir.ActivationFunctionType

    nc = bacc.Bacc(enable_partition_id=False)
    x16d = nc.declare_dram_parameter("x16", [P98, HWC], f16, isOutput=False)
    xsxd = nc.declare_dram_parameter("xsx", [P98, HWC], f16, isOutput=False)
    xsyd = nc.declare_dram_parameter("xsy", [P98, HWC], f16, isOutput=False)
    trwd = nc.declare_dram_parameter("trw", [2, HWC], f16, isOutput=False)
    spvd = nc.declare_dram_parameter("spv", [2, HWC], f16, isOutput=False)
    # all weights packed in one param: [0:98] wdx | [98:196] wdy |
    # [196:294] idsp (100 rows) | [294:485] bandw
    walld = nc.declare_dram_parameter("wall", [100, 485], f16, isOutput=False)
    out = nc.declare_dram_parameter("out", [2, HWC], f32, isOutput=True)

    XCH = 8                      # x upfront-load chunks
    XW = HWC // XCH

    with tile.TileContext(nc) as tc, ExitStack() as ctx:
        wpool = ctx.enter_context(tc.tile_pool(name="wpool", bufs=1))
        strp = ctx.enter_context(tc.tile_pool(name="strp", bufs=4))
        a12p = ctx.enter_context(tc.tile_pool(name="a12p", bufs=3))
        smp = ctx.enter_context(tc.tile_pool(name="smp", bufs=2))
        h2p = ctx.enter_context(tc.tile_pool(name="h2p", bufs=2))
        ktp = ctx.enter_context(tc.tile_pool(name="ktp", bufs=3))
        dns = ctx.enter_context(tc.tile_pool(name="dns", bufs=1))
        pS = ctx.enter_context(tc.tile_pool(name="pS", bufs=1, space="PSUM"))
        pacc = ctx.enter_context(tc.tile_pool(name="pacc", bufs=1, space="PSUM"))
        pred = ctx.enter_context(tc.tile_pool(name="pred", bufs=1, space="PSUM"))

        # one-time weights — ONE packed DMA via the Pool SWDGE queue: the
        # descriptor-gen overlaps the stream issues and its 0.3us transfer
        # slots between xsx0 and xsy0 on the DMA engines
        wallt = wpool.tile([100, 485], f16)
        nc.gpsimd.dma_start(out=wallt[:], in_=walld[:])
        wdxt = wallt[0:P98, 0:P98]
        wdyt = wallt[0:P98, P98:2 * P98]
        idspt = wallt[:, 2 * P98:2 * P98 + P98]
        bndt = wallt[0:P98, 294:485]

        RG = 4                  # quanta per streamed group
        NG = NT // RG
        PF = 2                  # group prefetch distance
        NTT = NT // 2           # super-tiles (2 quanta each)

        # PE warm-up: ~3us of dummy matmuls on a zeroed tile so the first
        # real A-matmuls run at the full-speed p-state (ramp needs 3us of
        # continuous PE activity), overlapping the initial stream DMAs
        zt = wpool.tile([P98, 64], f16, name="zwarm")
        nc.vector.memset(zt[:], 0.0)
        zp = pS.tile([P98, 4 * T], f32, tag="e12", name="zwarmp")
        for _ in range(30):
            nc.tensor.matmul(out=zp[0:64, 0:64], lhsT=zt[:, 0:64], rhs=zt[:],
                             start=True, stop=True)

        # streamed per-group tiles: xsx, xsy (dense), trow (49-bcast)
        xsxs, xsys, trws, h2gs = {}, {}, {}, {}

        def load_group_xy(g, halves=1):
            # spread issue cost across SEQ queues: xsx on SP, xsy on ACT.
            # halves=2 splits the transfers so the first A-matmuls can
            # start on sub-tile data (startup only).
            gx = strp.tile([P98, RG * T], f16, tag="xsx", name=f"xsx{g}")
            gy = strp.tile([P98, RG * T], f16, tag="xsy", name=f"xsy{g}")
            hw_ = RG * T // halves
            for h in range(halves):
                sl = slice(g * RG * T + h * hw_, g * RG * T + (h + 1) * hw_)
                ts = slice(h * hw_, (h + 1) * hw_)
                nc.sync.dma_start(out=gx[:, ts], in_=xsxd[:, sl])
                nc.scalar.dma_start(out=gy[:, ts], in_=xsyd[:, sl])
            xsxs[g], xsys[g] = gx, gy

        def load_group_rest(g):
            # trw + spv are consumed two iterations later than xsx/xsy
            sl = slice(g * RG * T, (g + 1) * RG * T)
            gt = strp.tile([P98, RG * T], f16, tag="trw", name=f"trw{g}")
            for b in range(2):
                nc.sync.dma_start(
                    out=gt[49 * b:49 * (b + 1), :],
                    in_=trwd[b, sl][None, :].to_broadcast((49, RG * T)))
            gh = h2p.tile([100, RG * T], f16, tag="h2", name=f"h2g{g}")
            nc.scalar.dma_start(out=gh[P98:100, :], in_=spvd[:, sl])
            trws[g], h2gs[g] = gt, gh

        def load_group(g):
            load_group_xy(g)
            load_group_rest(g)

        # x resident in SBUF as 8 chunk tiles (for the k*x product)
        xtiles = []

        def load_x_chunk():
            xi = len(xtiles)
            if xi >= XCH:
                return
            xt_ = wpool.tile([P98, XW], f16, name=f"xsb{xi}")
            nc.sync.dma_start(out=xt_[:], in_=x16d[:, xi * XW:(xi + 1) * XW])
            xtiles.append(xt_)

        def xs2(t0):
            # [98, 2T] slice covering quanta t0, t0+1 (same chunk: XW%2T==0)
            j = (t0 * T) // XW
            o = t0 * T - j * XW
            return xtiles[j][:, o:o + 2 * T]

        a12s, accs, kts, reds = {}, {}, {}, {}

        def stage_a(tt):
            """iter-u work: stream DMAs, A-matmuls(u), abs-drains(u)."""
            t0 = 2 * tt
            if t0 == 0:
                # g0, g1 first (needed by iters 0-3); chunk 0 defers to
                # iter 1 behind g2 (first stage_e needs it only at iter 3)
                load_group(0)
                load_group(1)
            elif tt % 2 == 1 and (tt + 3) // 2 < NG:
                # group g arrives at iter 2g-3 (A-matmuls need it at 2g)
                load_group((tt + 3) // 2)
                if tt == 1:
                    load_x_chunk()
            # chunk c streams in at iter 4c-2 (needed by stage_e at 4c+3)
            if tt >= 2 and tt % 4 == 2:
                load_x_chunk()
            g = t0 // RG
            go = (t0 % RG) * T
            a12 = a12p.tile([P98, 4 * T], f16, tag="a12")
            # e12 = [dxs0|dys0|dxs1|dys1]; ONE Abs drain with permuted out
            # AP writes a12 = [u0|u1|v0|v1]
            av = a12[:, :].rearrange("p (h q t) -> p q h t", h=2, q=2, t=T)
            e12 = pS.tile([P98, 4 * T], f32, tag="e12", name=f"e12_{tt}")
            for q in range(2):
                nc.tensor.matmul(out=e12[:, 2 * q * T:(2 * q + 1) * T],
                                 lhsT=wdxt[:],
                                 rhs=xsxs[g][:, go + q * T:go + (q + 1) * T],
                                 start=True, stop=True)
                nc.tensor.matmul(out=e12[:, (2 * q + 1) * T:(2 * q + 2) * T],
                                 lhsT=wdyt[:],
                                 rhs=xsys[g][:, go + q * T:go + (q + 1) * T],
                                 start=True, stop=True)
            nc.scalar.activation(out=av[:], in_=e12[:], func=Act.Abs,
                                 scale=1.0)
            if t0 % RG == RG - 2:
                del xsxs[g], xsys[g]
            a12s[tt] = a12

        dpre = {}

        def stage_b_d(tt):
            """just the d = u-v op of super-tile tt (pulled early into the
            last paced iteration's DVE slack to shorten the drain phase)."""
            a12 = a12s[tt]
            d = smp.tile([P98, 2 * T], f16, tag="d")
            nc.vector.tensor_tensor(out=d[:], in0=a12[:, 0:2 * T],
                                    in1=a12[:, 2 * T:4 * T], op=Alu.subtract)
            dpre[tt] = d

        def stage_b(tt):
            """DVE quad + acc matmuls for super-tile tt (lag 2)."""
            t0 = 2 * tt
            g = t0 // RG
            go = (t0 % RG) * T
            a12 = a12s.pop(tt)
            u2 = a12[:, 0:2 * T]
            v2 = a12[:, 2 * T:4 * T]
            if tt in dpre:
                d = dpre.pop(tt)
            else:
                d = smp.tile([P98, 2 * T], f16, tag="d")
                nc.vector.tensor_tensor(out=d[:], in0=u2, in1=v2,
                                        op=Alu.subtract)
            uv = smp.tile([P98, 2 * T], f16, tag="uv")
            nc.vector.tensor_tensor(out=uv[:], in0=u2, in1=v2, op=Alu.mult)
            dsq = smp.tile([P98, 2 * T], f16, tag="dsq")
            nc.vector.tensor_tensor(out=dsq[:], in0=d[:], in1=d[:], op=Alu.mult)
            h2 = h2gs[g]
            # spv rows 98:100 landed at group load; the stacked matmul on
            # [100, T] slices adds m*spv alongside the identity h2 add
            nc.vector.tensor_tensor(out=h2[0:P98, go:go + 2 * T], in0=uv[:],
                                    in1=trws[g][:, go:go + 2 * T], op=Alu.mult)
            if t0 % RG == RG - 2:
                del trws[g], h2gs[g]
            if tt >= NTT - 2:
                # last two super-tiles: no more drains — reuse the (dead)
                # e12 banks so these accs don't serialize on exp() freeing
                # the single regular acc bank
                if "accl" not in accs:
                    accs["accl"] = pS.tile([P98, 4 * T], f32, tag="e12",
                                           name="acclast")
                h = (tt - (NTT - 2)) * 2 * T
                acc = accs["accl"][:, h:h + 2 * T]
            else:
                acc = pacc.tile([P98, 2 * T], f32, tag="acc",
                                name=f"acc{tt}")
            for q in range(2):
                nc.tensor.matmul(out=acc[:, q * T:(q + 1) * T],
                                 lhsT=idspt[0:P98, :],
                                 rhs=dsq[:, q * T:(q + 1) * T],
                                 start=True, stop=False)
                nc.tensor.matmul(out=acc[:, q * T:(q + 1) * T],
                                 lhsT=idspt[:],
                                 rhs=h2[:, go + q * T:go + (q + 1) * T],
                                 start=False, stop=True)
            accs[tt] = acc

        def stage_e(tt):
            """exp + k*x for super-tile tt (lag 3) — emitted FIRST so the
            ACT stream runs exp before this iteration's drains."""
            acc = accs.pop(tt)
            ktw = ktp.tile([P98, 4 * T], f16, tag="ktw")
            nc.scalar.activation(out=ktw[:, 0:2 * T], in_=acc[:],
                                 func=Act.Exp, scale=-1.0)
            # last two super-tiles: w on DVE (fast) — the Pool mult's 2.1us
            # would sit exposed on the pipeline-drain critical path
            eng = nc.gpsimd if (WT_POOL and tt < NTT - 2) else nc.vector
            eng.tensor_tensor(out=ktw[:, 2 * T:4 * T], in0=ktw[:, 0:2 * T],
                              in1=xs2(2 * tt), op=Alu.mult)
            kts[tt] = ktw

        def stage_r(tt):
            """reduction matmuls for super-tile tt (lag 4): one 64-shift
            accumulation into a [128, T] PSUM pair (rows t / 64+t), no
            half boundaries."""
            t0 = 2 * tt
            ktw = kts.pop(tt)
            for q in range(2):
                t = t0 + q
                if t == 0:
                    reds["k"] = pred.tile([128, T], f32, tag="redk",
                                          name="redk")
                    reds["x"] = pred.tile([128, T], f32, tag="redx",
                                          name="redx")
                nc.tensor.matmul(out=reds["k"][:],
                                 lhsT=bndt[:, 63 - t:191 - t],
                                 rhs=ktw[:, q * T:(q + 1) * T],
                                 start=(t == 0), stop=(t == NT - 1),
                                 skip_group_check=True)
                nc.tensor.matmul(out=reds["x"][:],
                                 lhsT=bndt[:, 63 - t:191 - t],
                                 rhs=ktw[:, (2 + q) * T:(3 + q) * T],
                                 start=(t == 0), stop=(t == NT - 1),
                                 skip_group_check=True)

        def tail():
            """final normalize + store after all reduction matmuls."""
            redk = reds.pop("k")
            redx = reds.pop("x")
            rcp = dns.tile([128, T], f32, tag="rcp")
            # sum(k) in [1, 49]: far from approx-recip edge cases; ~51 ULP
            nc.vector.reciprocal_approx_fast(out=rcp[:], in_=redk[:])
            res = dns.tile([128, T], f32, tag="res")
            nc.vector.tensor_tensor(out=res[:], in0=redx[:], in1=rcp[:],
                                    op=Alu.mult)
            # res rows (b*64 + t) map 1:1 onto out[b, t*T:(t+1)*T] — one DMA
            ov = out[:, :].rearrange("b (t f) -> (b t) f", f=T)
            nc.sync.dma_start(out=ov, in_=res[:])

        WT_POOL = os.environ.get("KWPOOL", "1") == "1"
        DLT = float(os.environ.get("KD", "3.3"))  # us per super-iteration
        OFA = float(os.environ.get("KOA", "0.0"))
        OFB = float(os.environ.get("KOB", "0.3"))
        OFE = float(os.environ.get("KOE", "0.0"))
        OFR = float(os.environ.get("KOR", "0.6"))
        # emission order per iteration u: exp/w(u-3) first (ACT: exp
        # precedes drains(u)), then A(u), DVE+acc(u-2), red(u-4).
        # The pipeline drain (u >= NTT) is emitted densely in dep order.
        for u in range(NTT):
            pu = min(u, NTT - 1)
            if 3 <= u:
                with tc.tile_wait_until(pu * DLT + OFE, enable=DLT > 0):
                    stage_e(u - 3)
            with tc.tile_wait_until(pu * DLT + OFA, enable=DLT > 0):
                stage_a(u)
            if 2 <= u:
                with tc.tile_wait_until(pu * DLT + OFB, enable=DLT > 0):
                    stage_b(u - 2)
                    if u == NTT - 1:
                        # pull exp(NTT-3) + quad(NTT-2) into the last paced
                        # iteration: the drain phase's DVE quads otherwise
                        # serialize after the loop (exp frees the acc bank
                        # before the next stage_b allocates it)
                        stage_e(u - 2)
                        stage_b(u - 1)
            if 4 <= u:
                with tc.tile_wait_until(pu * DLT + OFR, enable=DLT > 0):
                    stage_r(u - 4)
        stage_e(NTT - 2)
        stage_r(NTT - 4)
        stage_b(NTT - 1)
        stage_r(NTT - 3)
        stage_e(NTT - 1)
        stage_r(NTT - 2)
        stage_r(NTT - 1)
        tail()

    if not nc.is_finalized():
        nc.finalize()
    return nc


def _run_pjrt(nc, in_maps):
    """Per-device single-core jits: this jax version's shard_map lowering
    emits multi-computation HLO that the bass_exec compile hook rejects,
    so dispatch one committed-args jit per NeuronCore instead (identical
    HLO -> the libneuronxla NEFF cache dedupes the 7 repeat compiles)."""
    import jax
    from jax import core as jcore
    from concourse import mybir
    from concourse.bass2jax import _bass_exec_p, install_neuronx_cc_hook

    install_neuronx_cc_hook()
    in_names, out_names, out_avals = [], [], []
    for alloc in nc.m.functions[0].allocations:
        if not isinstance(alloc, mybir.MemoryLocationSet):
            continue
        name = alloc.memorylocations[0].name
        if alloc.kind == "ExternalInput":
            in_names.append(name)
        elif alloc.kind == "ExternalOutput":
            out_avals.append(jcore.ShapedArray(
                tuple(alloc.tensor_shape), mybir.dt.np(alloc.dtype)))
            out_names.append(name)
    n_params = len(in_names)
    all_names = tuple(in_names) + tuple(out_names)
    donate = tuple(range(n_params, n_params + len(out_names)))

    def _body(*args):
        outs = _bass_exec_p.bind(
            *args, out_avals=tuple(out_avals), in_names=all_names,
            out_names=tuple(out_names), lowering_input_output_aliases=(),
            sim_require_finite=True, sim_require_nnan=True, nc=nc)
        return tuple(outs)

    fn = jax.jit(_body, donate_argnums=donate, keep_unused=True)
    devs = jax.devices()[:len(in_maps)]
    futs = []
    for c, m in enumerate(in_maps):
        args = [jax.device_put(np.ascontiguousarray(np.asarray(m[n])), devs[c])
                for n in in_names]
        args += [jax.device_put(np.zeros(a.shape, a.dtype), devs[c])
                 for a in out_avals]
        futs.append(fn(*args))
    jax.block_until_ready(futs)
    return [{name: np.asarray(f[i]) for i, name in enumerate(out_names)}
            for f in futs]


def prepare(x, sigx, sigy, theta, sigr):
    """Build (nc, in_maps) — shared by kernel() and test.py's profiler."""
    x = np.asarray(x, np.float32)
    sigx = np.asarray(sigx, np.float32)
    sigy = np.asarray(sigy, np.float32)
    theta = np.asarray(theta, np.float32)
    sigr = np.asarray(sigr, np.float32)

    if "nc" not in _compiled:
        _compiled["nc"] = _build_nc()
    nc = _compiled["nc"]

    wall = _build_weights()

    inv_sqrt2 = np.float32(1.0 / np.sqrt(2.0))
    trow = (2.0 - 2.0 * theta).astype(np.float32)
    spv = (0.5 / (sigr.astype(np.float64) ** 2)).astype(np.float32)

    wall16 = wall.astype(np.float16)

    in_maps = []
    for c in range(NCORES):
        rng = slice(c * HWC, (c + 1) * HWC)
        x_sh = np.ascontiguousarray(x[:, :, :, rng].reshape(P98, HWC))
        sx_sh = np.empty((P98, HWC), np.float32)
        sy_sh = np.empty((P98, HWC), np.float32)
        trw_sh = np.empty((2, HWC), np.float32)
        spv_sh = np.empty((2, HWC), np.float32)
        for b in range(2):
            gr = slice(b * HW + c * HWC, b * HW + (c + 1) * HWC)
            pr = slice(49 * b, 49 * (b + 1))
            sx_sh[pr] = x_sh[pr] * (sigx[gr] * inv_sqrt2)[None, :]
            sy_sh[pr] = x_sh[pr] * (sigy[gr] * inv_sqrt2)[None, :]
            trw_sh[b] = trow[gr]
            spv_sh[b] = spv[gr]
        in_maps.append({
            "x16": x_sh.astype(np.float16),
            "xsx": sx_sh.astype(np.float16),
            "xsy": sy_sh.astype(np.float16),
            "trw": trw_sh.astype(np.float16),
            "spv": spv_sh.astype(np.float16),
            "wall": wall16,
        })
    return nc, in_maps


def kernel(x, sigx, sigy, theta, sigr):
    nc, in_maps = prepare(x, sigx, sigy, theta, sigr)
    results = _run_pjrt(nc, in_maps)
    outs = [results[c]["out"] for c in range(NCORES)]
    return np.concatenate(outs, axis=1).astype(np.float32)
